# revision 17
# baseline (speedup 1.0000x reference)
"""CTDT memory updater on 8 Trainium2 NeuronCores.

Strategy (row-parallel scatter, per the node-id-range sharding):
  - The [100000, 256] memory table is sharded by node-id range across the 8
    cores (12500 rows each); weights are replicated. The host routes each
    step's (id, message) pairs to the owning core, so every step is fully
    local per core.
  - Chain routing: the host precomputes, for every (step, entry), where its
    input row value comes from: the original table (first touch of that
    row) or the output of the previous step that touched the same row.
    First-touch values are shipped from the host as a contiguous "ff"
    tensor (plain DMA, no descriptor-generation cost); repeat values are
    fetched with the SWDGE `dma_gather` custom op from a compact per-step
    output buffer `outs` in device DRAM.
  - Each step's entries are ordered [old-sourced | first-touch | recent
    (s-1)-sourced] and computed first-touch-first, so the only cross-step
    dependency (gather from outs[s-1]) resolves while independent tiles
    compute -> the PE pipeline runs continuously across steps.
  - The output table starts as a copy of the input shard; every step
    scatter-adds its per-entry delta (new - old value). Deltas telescope
    along each row's touch chain, so the final table value is exact up to
    fp32 rounding. Scatters run in the background (nothing reads the
    output table), off the critical path.
  - Matmul operands are rounded to bf16 (state, PSUM accumulation and the
    elementwise blend stay fp32): stationary-operand weight loads for
    4-byte dtypes are serial with matmuls on the PE and would dominate.

Layout notes:
  - Batch entry e = j*128 + p maps to tile j, SBUF partition p (dma_gather's
    native dst layout [128, T, elem]).
  - Gather/scatter indices are int16, wrapped in 16 partitions
    ([16, n/16] with token e at [e % 16, e // 16]) and replicated to all
    128 partitions (one copy per Q7 core).
  - Matmuls compute row-major [128b x 512] PSUM tiles: stationary operand is
    the activation chunk [k=128, b=128] (messages/dtdg pre-transposed on
    host; gathered memory rows transposed on PE), moving operand is a
    [128, 512] weight slice, plus a K=1 ones-row matmul that adds the bias.
  - gate/tanh pre-activations share one PSUM tile: columns 0:256 hold the
    sigmoid input, 256:512 the tanh input, with weight columns pre-arranged
    on the host so PSUM accumulation performs the i_C + h adds for free.
"""

import numpy as np

N_NODES = 100000
MEM = 256
N_CORES = 8
N_STEPS = 8
R = N_NODES // N_CORES  # rows per core
P = 128


def _build_program(cfg):
    import concourse.bacc as bacc
    import concourse.mybir as mybir
    import concourse.tile as tile

    f32 = mybir.dt.float32
    bf16 = mybir.dt.bfloat16
    i16 = mybir.dt.int16
    AF = mybir.ActivationFunctionType
    ALU = mybir.AluOpType

    OT, FT, RT = cfg  # per-step region tile counts
    TS = [OT[s] + FT[s] + RT[s] for s in range(N_STEPS)]
    C_MAX = max(TS) * P
    T_MAX = max(TS)
    ROWS = R + C_MAX
    NID_COLS = max((OT[s] + RT[s] + TS[s]) * 8 for s in range(N_STEPS))
    FF_ROWS = sum(FT) * P

    nc = bacc.Bacc("TRN2", target_bir_lowering=False, debug=False,
                   num_devices=N_CORES)

    tbl_in = nc.dram_tensor("tbl", [ROWS, MEM], f32, kind="ExternalInput").ap()
    ids_d = nc.dram_tensor("ids", [N_STEPS * P, NID_COLS], i16,
                           kind="ExternalInput").ap()
    ff_d = nc.dram_tensor("ff", [max(FF_ROWS, 1), MEM], f32,
                          kind="ExternalInput").ap()
    msg_d = nc.dram_tensor("msg", [N_STEPS * 2 * P, T_MAX * P], bf16,
                           kind="ExternalInput").ap()
    dt_d = nc.dram_tensor("dt", [2 * P, T_MAX * P], bf16,
                          kind="ExternalInput").ap()
    wc_d = nc.dram_tensor("wc", [2 * P, 512], bf16, kind="ExternalInput").ap()
    wh1_d = nc.dram_tensor("wh1", [2 * P, 512], bf16, kind="ExternalInput").ap()
    wh2_d = nc.dram_tensor("wh2", [2 * P, 512], bf16, kind="ExternalInput").ap()
    wd_d = nc.dram_tensor("wd", [2 * P, 512], bf16, kind="ExternalInput").ap()
    b1_d = nc.dram_tensor("b1", [1, 512], bf16, kind="ExternalInput").ap()
    b2_d = nc.dram_tensor("b2", [1, 512], bf16, kind="ExternalInput").ap()
    id_d = nc.dram_tensor("ident", [P, P], f32, kind="ExternalInput").ap()
    ones_d = nc.dram_tensor("ones1", [1, P], bf16, kind="ExternalInput").ap()
    tbl_out = nc.dram_tensor("out", [ROWS, MEM], f32, kind="ExternalOutput").ap()
    outs_d = nc.dram_tensor("outs", [N_STEPS * C_MAX, MEM], f32,
                            kind="Internal").ap()

    def chunks_of(lo, hi, max_tiles):
        n = max(1, -(-(hi - lo) // max_tiles))
        return [(lo + ((g * (hi - lo)) // n), lo + (((g + 1) * (hi - lo)) // n))
                for g in range(n)]

    with tile.TileContext(nc) as tc:
        with (
            tc.tile_pool(name="const", bufs=1) as const_pool,
            tc.tile_pool(name="ids", bufs=2) as ids_pool,
            tc.tile_pool(name="msgr", bufs=2) as msg_pool,
            tc.tile_pool(name="mg", bufs=2) as mg_pool,
            tc.tile_pool(name="new", bufs=2) as new_pool,
            tc.tile_pool(name="delta", bufs=2) as delta_pool,
            tc.tile_pool(name="work", bufs=3) as work_pool,
            tc.tile_pool(name="psum", bufs=2, space="PSUM") as psum_pool,
            tc.tile_pool(name="psumtr", bufs=2, space="PSUM") as psumtr_pool,
        ):
            # --- constants (all matmul operands arrive bf16 from the host) ---
            wc_sb = const_pool.tile([P, 1024], bf16)
            wh1_sb = const_pool.tile([P, 1024], bf16)
            wh2_sb = const_pool.tile([P, 1024], bf16)
            wd_sb = const_pool.tile([P, 1024], bf16)
            for sb, d in ((wc_sb, wc_d), (wh1_sb, wh1_d), (wh2_sb, wh2_d),
                          (wd_sb, wd_d)):
                for kc in range(2):
                    nc.sync.dma_start(out=sb[:, kc * 512:(kc + 1) * 512],
                                      in_=d[kc * P:(kc + 1) * P, :])
            b1_sb = const_pool.tile([1, 512], bf16)
            nc.sync.dma_start(out=b1_sb[:], in_=b1_d[:, :])
            b2_sb = const_pool.tile([1, 512], bf16)
            nc.sync.dma_start(out=b2_sb[:], in_=b2_d[:, :])
            ones_sb = const_pool.tile([1, P], bf16)
            nc.sync.dma_start(out=ones_sb[:], in_=ones_d[:, :])
            ident_sb = const_pool.tile([P, P], f32)
            nc.sync.dma_start(out=ident_sb[:], in_=id_d[:, :])

            # --- copy input shard -> output table (background) ---
            NCOPY = 8
            step_rows = (ROWS + NCOPY - 1) // NCOPY
            for i in range(NCOPY):
                a = i * step_rows
                b = min(ROWS, a + step_rows)
                nc.sync.dma_start(out=tbl_out[a:b, :], in_=tbl_in[a:b, :])

            ff_off = 0
            for s in range(N_STEPS):
                final = s == N_STEPS - 1
                T = TS[s]
                ot, ft, rt = OT[s], FT[s], RT[s]
                ids_sb = ids_pool.tile([P, NID_COLS], i16)
                nc.sync.dma_start(out=ids_sb[:],
                                  in_=ids_d[s * P:(s + 1) * P, :])
                msg_sb = msg_pool.tile([P, 2 * T_MAX * P], bf16)
                for kc in range(2):
                    row0 = (s * 2 + kc) * P
                    nc.sync.dma_start(
                        out=msg_sb[:, kc * T_MAX * P:kc * T_MAX * P + T * P],
                        in_=msg_d[row0:row0 + P, 0:T * P])
                if final:
                    dt_sb = msg_pool.tile([P, 2 * T_MAX * P], bf16, tag="dt",
                                          bufs=1)
                    for kc in range(2):
                        nc.sync.dma_start(
                            out=dt_sb[:, kc * T_MAX * P:kc * T_MAX * P + T * P],
                            in_=dt_d[kc * P:(kc + 1) * P, 0:T * P])

                mg3 = mg_pool.tile([P, T_MAX, MEM], f32)
                # first-touch values: contiguous load from host-packed ff
                if ft:
                    nc.sync.dma_start(
                        out=mg3[:, ot:ot + ft, :],
                        in_=ff_d[ff_off:ff_off + ft * P, :].rearrange(
                            "(t p) d -> p t d", p=P))
                    ff_off += ft * P
                # old-sourced values (steps < s-1): gather from outs
                if ot:
                    for j0, j1 in chunks_of(0, ot, 8):
                        n = (j1 - j0) * P
                        nc.gpsimd.dma_gather(
                            mg3[:, j0:j1, :], outs_d[:, :],
                            ids_sb[:, j0 * 8:j1 * 8], n, n, MEM)
                # recent values (step s-1): gather from outs
                if rt:
                    for j0, j1 in chunks_of(T - rt, T, 8):
                        n = (j1 - j0) * P
                        nc.gpsimd.dma_gather(
                            mg3[:, j0:j1, :], outs_d[:, :],
                            ids_sb[:, ot * 8 + (j0 - (T - rt)) * 8:
                                   ot * 8 + (j1 - (T - rt)) * 8], n, n, MEM)

                new3 = new_pool.tile([P, T_MAX, MEM], f32)
                del3 = delta_pool.tile([P, T_MAX, MEM], f32)

                # compute order: first-touch tiles first (no cross-step dep),
                # then old, then recent (gather has the longest to resolve)
                order = (list(range(ot, ot + ft)) + list(range(0, ot))
                         + list(range(ot + ft, T)))
                # chunk boundaries for outs writes / table scatters, in
                # compute order
                done_after = {}
                for ci, (o0, o1) in enumerate(chunks_of(0, T, 8)):
                    pos = max(order.index(j) for j in range(o0, o1))
                    done_after.setdefault(pos, []).append((o0, o1))

                for oi, j in enumerate(order):
                    mg_t = mg3[:, j, :]
                    tr_ps = psumtr_pool.tile([P, 2 * P], f32)
                    for kc in range(2):
                        nc.tensor.transpose(
                            out=tr_ps[:, kc * P:(kc + 1) * P],
                            in_=mg3[:, j, kc * P:(kc + 1) * P],
                            identity=ident_sb[:],
                        )
                    mgT = work_pool.tile([P, 2 * P], bf16)
                    nc.scalar.copy(mgT[:], tr_ps[:])

                    p1 = psum_pool.tile([P, 512], f32)
                    nc.tensor.matmul(p1[:], lhsT=ones_sb[:], rhs=b1_sb[:],
                                     start=True, stop=False)
                    for kc in range(2):
                        nc.tensor.matmul(
                            p1[:],
                            lhsT=msg_sb[:, kc * T_MAX * P + j * P:
                                        kc * T_MAX * P + (j + 1) * P],
                            rhs=wc_sb[:, kc * 512:(kc + 1) * 512],
                            start=False, stop=False)
                    for kc in range(2):
                        nc.tensor.matmul(
                            p1[:],
                            lhsT=mgT[:, kc * P:(kc + 1) * P],
                            rhs=wh1_sb[:, kc * 512:(kc + 1) * 512],
                            start=False, stop=kc == 1)
                    g = work_pool.tile([P, MEM], f32)
                    nc.scalar.activation(g[:], p1[:, 0:MEM], AF.Sigmoid)
                    hc = work_pool.tile([P, MEM], f32)
                    nc.scalar.activation(hc[:], p1[:, MEM:2 * MEM], AF.Tanh)

                    nw = new3[:, j, :]
                    dl = del3[:, j, :]
                    if not final:
                        # delta = (1 - g) * (hc - m_g) = (g - 1) * (m_g - hc)
                        d_t = work_pool.tile([P, MEM], f32)
                        nc.vector.tensor_sub(d_t[:], mg_t, hc[:])
                        nc.vector.scalar_tensor_tensor(
                            dl, g[:], 1.0, d_t[:],
                            op0=ALU.subtract, op1=ALU.mult)
                        nc.vector.tensor_add(nw, mg_t, dl)
                    else:
                        p2 = psum_pool.tile([P, 512], f32, tag="p2")
                        nc.tensor.matmul(p2[:], lhsT=ones_sb[:],
                                         rhs=b2_sb[:], start=True, stop=False)
                        for kc in range(2):
                            nc.tensor.matmul(
                                p2[:],
                                lhsT=dt_sb[:, kc * T_MAX * P + j * P:
                                           kc * T_MAX * P + (j + 1) * P],
                                rhs=wd_sb[:, kc * 512:(kc + 1) * 512],
                                start=False, stop=False)
                        for kc in range(2):
                            nc.tensor.matmul(
                                p2[:],
                                lhsT=mgT[:, kc * P:(kc + 1) * P],
                                rhs=wh2_sb[:, kc * 512:(kc + 1) * 512],
                                start=False, stop=kc == 1)
                        gd = work_pool.tile([P, MEM], f32)
                        nc.scalar.activation(gd[:], p2[:, 0:MEM], AF.Sigmoid)
                        hd = work_pool.tile([P, MEM], f32)
                        nc.scalar.activation(hd[:], p2[:, MEM:2 * MEM], AF.Tanh)
                        # delta = gd/2*(hd - m_g) + g/2*(hc - m_g)
                        d1 = work_pool.tile([P, MEM], f32, tag="d_t")
                        nc.vector.tensor_sub(d1[:], hd[:], mg_t)
                        t1 = work_pool.tile([P, MEM], f32)
                        nc.vector.scalar_tensor_tensor(
                            t1[:], gd[:], 0.5, d1[:],
                            op0=ALU.mult, op1=ALU.mult)
                        d2 = work_pool.tile([P, MEM], f32, tag="d_t")
                        nc.vector.tensor_sub(d2[:], hc[:], mg_t)
                        t2 = work_pool.tile([P, MEM], f32)
                        nc.vector.scalar_tensor_tensor(
                            t2[:], g[:], 0.5, d2[:],
                            op0=ALU.mult, op1=ALU.mult)
                        nc.vector.tensor_add(dl, t1[:], t2[:])
                        nc.vector.tensor_add(nw, mg_t, dl)

                    for o0, o1 in done_after.get(oi, ()):
                        # compact outs write (plain DMA, feeds later gathers)
                        a = s * C_MAX + o0 * P
                        b = s * C_MAX + o1 * P
                        nc.sync.dma_start(
                            out=outs_d[a:b, :].rearrange(
                                "(t p) d -> p t d", p=P),
                            in_=new3[:, o0:o1, :])
                        # background delta scatter into the output table
                        n = (o1 - o0) * P
                        nc.gpsimd.dma_scatter_add(
                            tbl_out[:, :], del3[:, o0:o1, :],
                            ids_sb[:, (ot + rt + o0) * 8:(ot + rt + o1) * 8],
                            n, n, MEM)
    nc.compile()
    return nc


def _prep_inputs(memory, node_ids, messages, dtdg_output,
                 W_C_w, W_C_b, W_D_w, W_D_b, W_h_w, W_h_b):
    import ml_dtypes

    bf16 = ml_dtypes.bfloat16
    node_ids = np.asarray(node_ids)
    owner = node_ids // R  # [8, B]

    # --- route batches to cores and classify each entry's value source ---
    percore = []
    for c in range(N_CORES):
        steps = []
        last_s = np.full(R, -1, np.int64)  # last step that touched each row
        for s in range(N_STEPS):
            sel = np.nonzero(owner[s] == c)[0]
            loc = (node_ids[s, sel] - c * R).astype(np.int64)
            prev_s = last_s[loc]
            is_ff = prev_s < 0
            is_rec = (~is_ff) & (prev_s == s - 1)
            is_old = (~is_ff) & (~is_rec)
            steps.append(dict(sel=sel, loc=loc,
                              o=np.nonzero(is_old)[0],
                              f=np.nonzero(is_ff)[0],
                              r=np.nonzero(is_rec)[0]))
            last_s[loc] = s
        percore.append(steps)

    # global region tile counts
    OT, FT, RT = [], [], []
    for s in range(N_STEPS):
        OT.append(max(-(-len(percore[c][s]["o"]) // P) for c in range(N_CORES)))
        FT.append(max(-(-len(percore[c][s]["f"]) // P) for c in range(N_CORES)))
        RT.append(max(-(-len(percore[c][s]["r"]) // P) for c in range(N_CORES)))
    cfg = (tuple(OT), tuple(FT), tuple(RT))
    TS = [OT[s] + FT[s] + RT[s] for s in range(N_STEPS)]
    C_MAX = max(TS) * P
    T_MAX = max(TS)
    NID_COLS = max((OT[s] + RT[s] + TS[s]) * 8 for s in range(N_STEPS))
    FF_ROWS = sum(FT) * P

    WcT = np.ascontiguousarray(W_C_w.T)
    WhT = W_h_w.T
    wh1 = np.ascontiguousarray(np.concatenate(
        [WhT[:, 256:512], WhT[:, 768:1024]], axis=1))
    wh2 = np.ascontiguousarray(np.concatenate(
        [WhT[:, 0:256], WhT[:, 512:768]], axis=1))
    WdT = np.ascontiguousarray(W_D_w.T)
    b1 = np.concatenate([W_C_b[:256] + W_h_b[256:512],
                         W_C_b[256:512] + W_h_b[768:1024]])[None, :]
    b2 = np.concatenate([W_D_b[:256] + W_h_b[0:256],
                         W_D_b[256:512] + W_h_b[512:768]])[None, :]

    def fold(block, T):
        # [T*128, 256] row-major entries -> [2*128, T*128] transposed k-major
        out = np.empty((2 * P, T_MAX * P), np.float32)
        for kc in range(2):
            b3 = block[:, kc * P:(kc + 1) * P].reshape(T, P, P)  # [j, b, k]
            out[kc * P:(kc + 1) * P, :T * P] = np.ascontiguousarray(
                b3.transpose(2, 0, 1)).reshape(P, T * P)
        return out

    def wrap16(tokens):
        # [n] -> [128, n/16] int16, token e at [e%16, e//16], replicated
        n = tokens.size
        blk = tokens.astype(np.int16).reshape(n // 16, 16).T
        return np.tile(blk, (8, 1))

    in_maps = []
    for c in range(N_CORES):
        tbl = np.zeros((R + C_MAX, MEM), np.float32)
        tbl[:R] = memory[c * R:(c + 1) * R]
        ids_arr = np.zeros((N_STEPS * P, NID_COLS), np.int16)
        msg_arr = np.zeros((N_STEPS * 2 * P, T_MAX * P), bf16)
        dt_arr = np.zeros((2 * P, T_MAX * P), bf16)
        ff_arr = np.zeros((max(FF_ROWS, 1), MEM), np.float32)
        ff_off = 0
        # flat outs position of each (step, entry) for chain bookkeeping
        pos_of = np.full(R, -1, np.int64)
        for s in range(N_STEPS):
            st = percore[c][s]
            T = TS[s]
            C = T * P
            ot, ft, rt = OT[s], FT[s], RT[s]
            n_o, n_f, n_r = len(st["o"]), len(st["f"]), len(st["r"])

            # entry order: [old pad ot*P][ff pad ft*P][recent pad rt*P]
            perm = np.full(C, -1, np.int64)  # batch-item index per entry slot
            perm[0:n_o] = st["o"]
            perm[ot * P:ot * P + n_f] = st["f"]
            perm[(ot + ft) * P:(ot + ft) * P + n_r] = st["r"]
            valid = perm >= 0

            # gather tokens: flat outs index of each entry's source value
            if ot:
                tok = np.zeros(ot * P, np.int64)
                tok[:n_o] = pos_of[st["loc"][st["o"]]]
                assert n_o == 0 or ((tok[:n_o] >= 0).all()
                                    and (tok[:n_o] < (s - 1) * C_MAX).all())
                ids_arr[s * P:(s + 1) * P, 0:ot * 8] = wrap16(tok)
            if rt:
                tok = np.zeros(rt * P, np.int64)
                tok[:n_r] = pos_of[st["loc"][st["r"]]]
                assert n_r == 0 or ((tok[:n_r] >= (s - 1) * C_MAX).all()
                                    and (tok[:n_r] < s * C_MAX).all())
                ids_arr[s * P:(s + 1) * P, ot * 8:(ot + rt) * 8] = wrap16(tok)
            # scatter tokens: local table row per entry; pads -> scratch rows
            rows = np.empty(C, np.int64)
            rows[valid] = st["loc"][perm[valid]]
            rows[~valid] = R + np.arange((~valid).sum())
            ids_arr[s * P:(s + 1) * P,
                    (ot + rt) * 8:(ot + rt + T) * 8] = wrap16(rows)

            # ff values
            if ft:
                ffv = np.zeros((ft * P, MEM), np.float32)
                ffv[:n_f] = memory[c * R + st["loc"][st["f"]]]
                ff_arr[ff_off:ff_off + ft * P] = ffv
                ff_off += ft * P

            # messages / dtdg in entry order
            pm = np.zeros((C, MEM), np.float32)
            pm[valid] = messages[s, st["sel"][perm[valid]]]
            msg_arr[s * 2 * P:(s + 1) * 2 * P] = fold(pm, T).astype(bf16)
            if s == N_STEPS - 1:
                pd = np.zeros((C, MEM), np.float32)
                pd[valid] = dtdg_output[node_ids[s, st["sel"][perm[valid]]]]
                dt_arr[:] = fold(pd, T).astype(bf16)

            # record flat outs positions for the next steps' chains
            pos_of[st["loc"][perm[valid]]] = s * C_MAX + np.nonzero(valid)[0]

        in_maps.append(dict(
            tbl=tbl, ids=ids_arr, ff=ff_arr, msg=msg_arr, dt=dt_arr,
            wc=WcT.astype(bf16), wh1=wh1.astype(bf16),
            wh2=wh2.astype(bf16), wd=WdT.astype(bf16),
            b1=b1.astype(bf16), b2=b2.astype(bf16),
            ident=np.eye(P, dtype=np.float32),
            ones1=np.ones((1, P), bf16),
        ))
    return in_maps, cfg


_CACHE = {}


def run_sharded(in_maps, cfg, trace=False, **kw):
    from concourse.bass_utils import run_bass_kernel_spmd

    if cfg not in _CACHE:
        _CACHE[cfg] = _build_program(cfg)
    nc = _CACHE[cfg]
    return run_bass_kernel_spmd(nc, in_maps, core_ids=list(range(N_CORES)),
                                trace=trace, **kw)


def kernel(memory, node_ids, messages, dtdg_output,
           W_C_w, W_C_b, W_D_w, W_D_b, W_h_w, W_h_b):
    memory = np.asarray(memory, np.float32)
    messages = np.asarray(messages, np.float32)
    dtdg_output = np.asarray(dtdg_output, np.float32)
    in_maps, cfg = _prep_inputs(memory, node_ids, messages, dtdg_output,
                                np.asarray(W_C_w, np.float32),
                                np.asarray(W_C_b, np.float32),
                                np.asarray(W_D_w, np.float32),
                                np.asarray(W_D_b, np.float32),
                                np.asarray(W_h_w, np.float32),
                                np.asarray(W_h_b, np.float32))
    res = run_sharded(in_maps, cfg)
    out = np.empty((N_NODES, MEM), np.float32)
    for c in range(N_CORES):
        out[c * R:(c + 1) * R] = res.results[c]["out"][:R]
    return out


# revision 18
# speedup vs baseline: 1.5891x; 1.5891x over previous
"""CTDT memory updater on 8 Trainium2 NeuronCores.

Strategy (row-parallel scatter, per the node-id-range sharding):
  - The [100000, 256] memory table is sharded by node-id range across the 8
    cores (12500 rows each); weights are replicated. The host routes each
    step's (id, message) pairs to the owning core, so every step is fully
    local per core.
  - Chain routing: the host precomputes, for every (step, entry), where its
    input row value comes from: the original table (first touch of that
    row) or the output of the previous step that touched the same row.
    First-touch values are shipped from the host as a contiguous "ff"
    tensor (plain DMA, no descriptor-generation cost); repeat values are
    fetched with the SWDGE `dma_gather` custom op from a compact per-step
    output buffer `outs` in device DRAM.
  - Each step's entries are ordered [old-sourced | first-touch | recent
    (s-1)-sourced] and computed first-touch-first, so the only cross-step
    dependency (gather from outs[s-1]) resolves while independent tiles
    compute -> the PE pipeline runs continuously across steps.
  - The output table starts as a copy of the input shard; every step
    scatter-adds its per-entry delta (new - old value). Deltas telescope
    along each row's touch chain, so the final table value is exact up to
    fp32 rounding. Scatters run in the background (nothing reads the
    output table), off the critical path.
  - Matmul operands are rounded to bf16 (state, PSUM accumulation and the
    elementwise blend stay fp32): stationary-operand weight loads for
    4-byte dtypes are serial with matmuls on the PE and would dominate.

Layout notes:
  - Batch entry e = j*128 + p maps to tile j, SBUF partition p (dma_gather's
    native dst layout [128, T, elem]).
  - Gather/scatter indices are int16, wrapped in 16 partitions
    ([16, n/16] with token e at [e % 16, e // 16]) and replicated to all
    128 partitions (one copy per Q7 core).
  - Matmuls compute row-major [128b x 512] PSUM tiles: stationary operand is
    the activation chunk [k=128, b=128] (messages/dtdg pre-transposed on
    host; gathered memory rows transposed on PE), moving operand is a
    [128, 512] weight slice, plus a K=1 ones-row matmul that adds the bias.
  - gate/tanh pre-activations share one PSUM tile: columns 0:256 hold the
    sigmoid input, 256:512 the tanh input, with weight columns pre-arranged
    on the host so PSUM accumulation performs the i_C + h adds for free.
"""

import numpy as np

N_NODES = 100000
MEM = 256
N_CORES = 8
N_STEPS = 8
R = N_NODES // N_CORES  # rows per core
P = 128


def _build_program(cfg):
    import concourse.bacc as bacc
    import concourse.mybir as mybir
    import concourse.tile as tile

    f32 = mybir.dt.float32
    bf16 = mybir.dt.bfloat16
    i16 = mybir.dt.int16
    AF = mybir.ActivationFunctionType
    ALU = mybir.AluOpType

    OT, FT, RT = cfg  # per-step region tile counts
    TS = [OT[s] + FT[s] + RT[s] for s in range(N_STEPS)]
    C_MAX = max(TS) * P
    T_MAX = max(TS)
    NID_COLS = max((OT[s] + RT[s]) * 8 for s in range(N_STEPS)) or 8
    FF_ROWS = sum(FT) * P

    nc = bacc.Bacc("TRN2", target_bir_lowering=False, debug=False,
                   num_devices=N_CORES)

    ids_d = nc.dram_tensor("ids", [N_STEPS * P, NID_COLS], i16,
                           kind="ExternalInput").ap()
    ff_d = nc.dram_tensor("ff", [max(FF_ROWS, 1), MEM], f32,
                          kind="ExternalInput").ap()
    msg_d = nc.dram_tensor("msg", [N_STEPS * 2 * P, T_MAX * P], bf16,
                           kind="ExternalInput").ap()
    dt_d = nc.dram_tensor("dt", [2 * P, T_MAX * P], bf16,
                          kind="ExternalInput").ap()
    wc_d = nc.dram_tensor("wc", [2 * P, 512], bf16, kind="ExternalInput").ap()
    wh1_d = nc.dram_tensor("wh1", [2 * P, 512], bf16, kind="ExternalInput").ap()
    wh2_d = nc.dram_tensor("wh2", [2 * P, 512], bf16, kind="ExternalInput").ap()
    wd_d = nc.dram_tensor("wd", [2 * P, 512], bf16, kind="ExternalInput").ap()
    b1_d = nc.dram_tensor("b1", [1, 512], bf16, kind="ExternalInput").ap()
    b2_d = nc.dram_tensor("b2", [1, 512], bf16, kind="ExternalInput").ap()
    id_d = nc.dram_tensor("ident", [P, P], f32, kind="ExternalInput").ap()
    ones_d = nc.dram_tensor("ones1", [1, P], bf16, kind="ExternalInput").ap()
    outs_d = nc.dram_tensor("outs", [N_STEPS * C_MAX, MEM], f32,
                            kind="ExternalOutput").ap()

    def chunks_of(lo, hi, max_tiles):
        n = max(1, -(-(hi - lo) // max_tiles))
        return [(lo + ((g * (hi - lo)) // n), lo + (((g + 1) * (hi - lo)) // n))
                for g in range(n)]

    with tile.TileContext(nc) as tc:
        with (
            tc.tile_pool(name="const", bufs=1) as const_pool,
            tc.tile_pool(name="ids", bufs=2) as ids_pool,
            tc.tile_pool(name="msgr", bufs=2) as msg_pool,
            tc.tile_pool(name="mg", bufs=2) as mg_pool,
            tc.tile_pool(name="new", bufs=2) as new_pool,
            tc.tile_pool(name="work", bufs=3) as work_pool,
            tc.tile_pool(name="psum", bufs=2, space="PSUM") as psum_pool,
            tc.tile_pool(name="psumtr", bufs=2, space="PSUM") as psumtr_pool,
        ):
            # --- constants (all matmul operands arrive bf16 from the host) ---
            wc_sb = const_pool.tile([P, 1024], bf16)
            wh1_sb = const_pool.tile([P, 1024], bf16)
            wh2_sb = const_pool.tile([P, 1024], bf16)
            wd_sb = const_pool.tile([P, 1024], bf16)
            for sb, d in ((wc_sb, wc_d), (wh1_sb, wh1_d), (wh2_sb, wh2_d),
                          (wd_sb, wd_d)):
                for kc in range(2):
                    nc.sync.dma_start(out=sb[:, kc * 512:(kc + 1) * 512],
                                      in_=d[kc * P:(kc + 1) * P, :])
            b1_sb = const_pool.tile([1, 512], bf16)
            nc.sync.dma_start(out=b1_sb[:], in_=b1_d[:, :])
            b2_sb = const_pool.tile([1, 512], bf16)
            nc.sync.dma_start(out=b2_sb[:], in_=b2_d[:, :])
            ones_sb = const_pool.tile([1, P], bf16)
            nc.sync.dma_start(out=ones_sb[:], in_=ones_d[:, :])
            ident_sb = const_pool.tile([P, P], f32)
            nc.sync.dma_start(out=ident_sb[:], in_=id_d[:, :])

            ff_off = 0
            for s in range(N_STEPS):
                final = s == N_STEPS - 1
                T = TS[s]
                ot, ft, rt = OT[s], FT[s], RT[s]
                ids_sb = ids_pool.tile([P, NID_COLS], i16)
                nc.sync.dma_start(out=ids_sb[:],
                                  in_=ids_d[s * P:(s + 1) * P, :])
                msg_sb = msg_pool.tile([P, 2 * T_MAX * P], bf16)
                for kc in range(2):
                    row0 = (s * 2 + kc) * P
                    nc.sync.dma_start(
                        out=msg_sb[:, kc * T_MAX * P:kc * T_MAX * P + T * P],
                        in_=msg_d[row0:row0 + P, 0:T * P])
                if final:
                    dt_sb = msg_pool.tile([P, 2 * T_MAX * P], bf16, tag="dt",
                                          bufs=1)
                    for kc in range(2):
                        nc.sync.dma_start(
                            out=dt_sb[:, kc * T_MAX * P:kc * T_MAX * P + T * P],
                            in_=dt_d[kc * P:(kc + 1) * P, 0:T * P])

                mg3 = mg_pool.tile([P, T_MAX, MEM], f32)
                # first-touch values: contiguous load from host-packed ff
                if ft:
                    nc.sync.dma_start(
                        out=mg3[:, ot:ot + ft, :],
                        in_=ff_d[ff_off:ff_off + ft * P, :].rearrange(
                            "(t p) d -> p t d", p=P))
                    ff_off += ft * P
                # old-sourced values (steps < s-1): gather from outs
                if ot:
                    for j0, j1 in chunks_of(0, ot, 8):
                        n = (j1 - j0) * P
                        nc.gpsimd.dma_gather(
                            mg3[:, j0:j1, :], outs_d[:, :],
                            ids_sb[:, j0 * 8:j1 * 8], n, n, MEM)
                # recent values (step s-1): gather from outs
                if rt:
                    for j0, j1 in chunks_of(T - rt, T, 8):
                        n = (j1 - j0) * P
                        nc.gpsimd.dma_gather(
                            mg3[:, j0:j1, :], outs_d[:, :],
                            ids_sb[:, ot * 8 + (j0 - (T - rt)) * 8:
                                   ot * 8 + (j1 - (T - rt)) * 8], n, n, MEM)

                new3 = new_pool.tile([P, T_MAX, MEM], f32)

                # compute order: first-touch tiles first (no cross-step dep),
                # then old, then recent (gather has the longest to resolve)
                order = (list(range(ot, ot + ft)) + list(range(0, ot))
                         + list(range(ot + ft, T)))
                # chunk boundaries for outs writes / table scatters, in
                # compute order
                done_after = {}
                for ci, (o0, o1) in enumerate(chunks_of(0, T, 8)):
                    pos = max(order.index(j) for j in range(o0, o1))
                    done_after.setdefault(pos, []).append((o0, o1))

                for oi, j in enumerate(order):
                    mg_t = mg3[:, j, :]
                    tr_ps = psumtr_pool.tile([P, 2 * P], f32)
                    for kc in range(2):
                        nc.tensor.transpose(
                            out=tr_ps[:, kc * P:(kc + 1) * P],
                            in_=mg3[:, j, kc * P:(kc + 1) * P],
                            identity=ident_sb[:],
                        )
                    mgT = work_pool.tile([P, 2 * P], bf16)
                    nc.scalar.copy(mgT[:], tr_ps[:])

                    p1 = psum_pool.tile([P, 512], f32)
                    nc.tensor.matmul(p1[:], lhsT=ones_sb[:], rhs=b1_sb[:],
                                     start=True, stop=False)
                    for kc in range(2):
                        nc.tensor.matmul(
                            p1[:],
                            lhsT=msg_sb[:, kc * T_MAX * P + j * P:
                                        kc * T_MAX * P + (j + 1) * P],
                            rhs=wc_sb[:, kc * 512:(kc + 1) * 512],
                            start=False, stop=False)
                    for kc in range(2):
                        nc.tensor.matmul(
                            p1[:],
                            lhsT=mgT[:, kc * P:(kc + 1) * P],
                            rhs=wh1_sb[:, kc * 512:(kc + 1) * 512],
                            start=False, stop=kc == 1)
                    g = work_pool.tile([P, MEM], f32)
                    nc.scalar.activation(g[:], p1[:, 0:MEM], AF.Sigmoid)
                    hc = work_pool.tile([P, MEM], f32)
                    nc.scalar.activation(hc[:], p1[:, MEM:2 * MEM], AF.Tanh)

                    nw = new3[:, j, :]
                    if not final:
                        # new = hc + g * (m_g - hc)
                        d_t = work_pool.tile([P, MEM], f32)
                        nc.vector.tensor_sub(d_t[:], mg_t, hc[:])
                        e_t = work_pool.tile([P, MEM], f32)
                        nc.vector.tensor_mul(e_t[:], g[:], d_t[:])
                        nc.vector.tensor_add(nw, hc[:], e_t[:])
                    else:
                        p2 = psum_pool.tile([P, 512], f32, tag="p2")
                        nc.tensor.matmul(p2[:], lhsT=ones_sb[:],
                                         rhs=b2_sb[:], start=True, stop=False)
                        for kc in range(2):
                            nc.tensor.matmul(
                                p2[:],
                                lhsT=dt_sb[:, kc * T_MAX * P + j * P:
                                           kc * T_MAX * P + (j + 1) * P],
                                rhs=wd_sb[:, kc * 512:(kc + 1) * 512],
                                start=False, stop=False)
                        for kc in range(2):
                            nc.tensor.matmul(
                                p2[:],
                                lhsT=mgT[:, kc * P:(kc + 1) * P],
                                rhs=wh2_sb[:, kc * 512:(kc + 1) * 512],
                                start=False, stop=kc == 1)
                        gd = work_pool.tile([P, MEM], f32)
                        nc.scalar.activation(gd[:], p2[:, 0:MEM], AF.Sigmoid)
                        hd = work_pool.tile([P, MEM], f32)
                        nc.scalar.activation(hd[:], p2[:, MEM:2 * MEM], AF.Tanh)
                        # new = m_g + gd/2*(hd - m_g) + g/2*(hc - m_g)
                        d1 = work_pool.tile([P, MEM], f32, tag="d_t")
                        nc.vector.tensor_sub(d1[:], hd[:], mg_t)
                        t1 = work_pool.tile([P, MEM], f32)
                        nc.vector.scalar_tensor_tensor(
                            t1[:], gd[:], 0.5, d1[:],
                            op0=ALU.mult, op1=ALU.mult)
                        d2 = work_pool.tile([P, MEM], f32, tag="d_t")
                        nc.vector.tensor_sub(d2[:], hc[:], mg_t)
                        t2 = work_pool.tile([P, MEM], f32)
                        nc.vector.scalar_tensor_tensor(
                            t2[:], g[:], 0.5, d2[:],
                            op0=ALU.mult, op1=ALU.mult)
                        acc = work_pool.tile([P, MEM], f32)
                        nc.vector.tensor_add(acc[:], t1[:], t2[:])
                        nc.vector.tensor_add(nw, mg_t, acc[:])

                    for o0, o1 in done_after.get(oi, ()):
                        # compact outs write (plain DMA, feeds later gathers
                        # and the host-side final placement)
                        a = s * C_MAX + o0 * P
                        b = s * C_MAX + o1 * P
                        nc.sync.dma_start(
                            out=outs_d[a:b, :].rearrange(
                                "(t p) d -> p t d", p=P),
                            in_=new3[:, o0:o1, :])
    nc.compile()
    return nc


def _prep_inputs(memory, node_ids, messages, dtdg_output,
                 W_C_w, W_C_b, W_D_w, W_D_b, W_h_w, W_h_b):
    import ml_dtypes

    bf16 = ml_dtypes.bfloat16
    node_ids = np.asarray(node_ids)
    owner = node_ids // R  # [8, B]

    # --- route batches to cores and classify each entry's value source ---
    percore = []
    for c in range(N_CORES):
        steps = []
        last_s = np.full(R, -1, np.int64)  # last step that touched each row
        for s in range(N_STEPS):
            sel = np.nonzero(owner[s] == c)[0]
            loc = (node_ids[s, sel] - c * R).astype(np.int64)
            prev_s = last_s[loc]
            is_ff = prev_s < 0
            is_rec = (~is_ff) & (prev_s == s - 1)
            is_old = (~is_ff) & (~is_rec)
            steps.append(dict(sel=sel, loc=loc,
                              o=np.nonzero(is_old)[0],
                              f=np.nonzero(is_ff)[0],
                              r=np.nonzero(is_rec)[0]))
            last_s[loc] = s
        percore.append(steps)

    # global region tile counts
    OT, FT, RT = [], [], []
    for s in range(N_STEPS):
        OT.append(max(-(-len(percore[c][s]["o"]) // P) for c in range(N_CORES)))
        FT.append(max(-(-len(percore[c][s]["f"]) // P) for c in range(N_CORES)))
        RT.append(max(-(-len(percore[c][s]["r"]) // P) for c in range(N_CORES)))
    cfg = (tuple(OT), tuple(FT), tuple(RT))
    TS = [OT[s] + FT[s] + RT[s] for s in range(N_STEPS)]
    C_MAX = max(TS) * P
    T_MAX = max(TS)
    NID_COLS = max((OT[s] + RT[s]) * 8 for s in range(N_STEPS)) or 8
    FF_ROWS = sum(FT) * P

    WcT = np.ascontiguousarray(W_C_w.T)
    WhT = W_h_w.T
    wh1 = np.ascontiguousarray(np.concatenate(
        [WhT[:, 256:512], WhT[:, 768:1024]], axis=1))
    wh2 = np.ascontiguousarray(np.concatenate(
        [WhT[:, 0:256], WhT[:, 512:768]], axis=1))
    WdT = np.ascontiguousarray(W_D_w.T)
    b1 = np.concatenate([W_C_b[:256] + W_h_b[256:512],
                         W_C_b[256:512] + W_h_b[768:1024]])[None, :]
    b2 = np.concatenate([W_D_b[:256] + W_h_b[0:256],
                         W_D_b[256:512] + W_h_b[512:768]])[None, :]

    def fold(block, T):
        # [T*128, 256] row-major entries -> [2*128, T*128] transposed k-major
        out = np.empty((2 * P, T_MAX * P), np.float32)
        for kc in range(2):
            b3 = block[:, kc * P:(kc + 1) * P].reshape(T, P, P)  # [j, b, k]
            out[kc * P:(kc + 1) * P, :T * P] = np.ascontiguousarray(
                b3.transpose(2, 0, 1)).reshape(P, T * P)
        return out

    def wrap16(tokens):
        # [n] -> [128, n/16] int16, token e at [e%16, e//16], replicated
        n = tokens.size
        blk = tokens.astype(np.int16).reshape(n // 16, 16).T
        return np.tile(blk, (8, 1))

    in_maps = []
    place = []   # per core: (touched_local_rows, outs_flat_positions)
    for c in range(N_CORES):
        ids_arr = np.zeros((N_STEPS * P, NID_COLS), np.int16)
        msg_arr = np.zeros((N_STEPS * 2 * P, T_MAX * P), bf16)
        dt_arr = np.zeros((2 * P, T_MAX * P), bf16)
        ff_arr = np.zeros((max(FF_ROWS, 1), MEM), np.float32)
        ff_off = 0
        # flat outs position of each (step, entry) for chain bookkeeping
        pos_of = np.full(R, -1, np.int64)
        for s in range(N_STEPS):
            st = percore[c][s]
            T = TS[s]
            C = T * P
            ot, ft, rt = OT[s], FT[s], RT[s]
            n_o, n_f, n_r = len(st["o"]), len(st["f"]), len(st["r"])

            # entry order: [old pad ot*P][ff pad ft*P][recent pad rt*P]
            perm = np.full(C, -1, np.int64)  # batch-item index per entry slot
            perm[0:n_o] = st["o"]
            perm[ot * P:ot * P + n_f] = st["f"]
            perm[(ot + ft) * P:(ot + ft) * P + n_r] = st["r"]
            valid = perm >= 0

            # gather tokens: flat outs index of each entry's source value
            if ot:
                tok = np.zeros(ot * P, np.int64)
                tok[:n_o] = pos_of[st["loc"][st["o"]]]
                assert n_o == 0 or ((tok[:n_o] >= 0).all()
                                    and (tok[:n_o] < (s - 1) * C_MAX).all())
                ids_arr[s * P:(s + 1) * P, 0:ot * 8] = wrap16(tok)
            if rt:
                tok = np.zeros(rt * P, np.int64)
                tok[:n_r] = pos_of[st["loc"][st["r"]]]
                assert n_r == 0 or ((tok[:n_r] >= (s - 1) * C_MAX).all()
                                    and (tok[:n_r] < s * C_MAX).all())
                ids_arr[s * P:(s + 1) * P, ot * 8:(ot + rt) * 8] = wrap16(tok)
            # ff values
            if ft:
                ffv = np.zeros((ft * P, MEM), np.float32)
                ffv[:n_f] = memory[c * R + st["loc"][st["f"]]]
                ff_arr[ff_off:ff_off + ft * P] = ffv
                ff_off += ft * P

            # messages / dtdg in entry order
            pm = np.zeros((C, MEM), np.float32)
            pm[valid] = messages[s, st["sel"][perm[valid]]]
            msg_arr[s * 2 * P:(s + 1) * 2 * P] = fold(pm, T).astype(bf16)
            if s == N_STEPS - 1:
                pd = np.zeros((C, MEM), np.float32)
                pd[valid] = dtdg_output[node_ids[s, st["sel"][perm[valid]]]]
                dt_arr[:] = fold(pd, T).astype(bf16)

            # record flat outs positions for the next steps' chains
            pos_of[st["loc"][perm[valid]]] = s * C_MAX + np.nonzero(valid)[0]

        touched = np.nonzero(pos_of >= 0)[0]
        place.append((touched, pos_of[touched]))
        in_maps.append(dict(
            ids=ids_arr, ff=ff_arr, msg=msg_arr, dt=dt_arr,
            wc=WcT.astype(bf16), wh1=wh1.astype(bf16),
            wh2=wh2.astype(bf16), wd=WdT.astype(bf16),
            b1=b1.astype(bf16), b2=b2.astype(bf16),
            ident=np.eye(P, dtype=np.float32),
            ones1=np.ones((1, P), bf16),
        ))
    return in_maps, cfg, place


_CACHE = {}


def run_sharded(in_maps, cfg, trace=False, **kw):
    from concourse.bass_utils import run_bass_kernel_spmd

    if cfg not in _CACHE:
        _CACHE[cfg] = _build_program(cfg)
    nc = _CACHE[cfg]
    return run_bass_kernel_spmd(nc, in_maps, core_ids=list(range(N_CORES)),
                                trace=trace, **kw)


def kernel(memory, node_ids, messages, dtdg_output,
           W_C_w, W_C_b, W_D_w, W_D_b, W_h_w, W_h_b):
    memory = np.asarray(memory, np.float32)
    messages = np.asarray(messages, np.float32)
    dtdg_output = np.asarray(dtdg_output, np.float32)
    in_maps, cfg, place = _prep_inputs(memory, node_ids, messages,
                                       dtdg_output,
                                       np.asarray(W_C_w, np.float32),
                                       np.asarray(W_C_b, np.float32),
                                       np.asarray(W_D_w, np.float32),
                                       np.asarray(W_D_b, np.float32),
                                       np.asarray(W_h_w, np.float32),
                                       np.asarray(W_h_b, np.float32))
    res = run_sharded(in_maps, cfg)
    out = np.array(memory, np.float32, copy=True)
    for c in range(N_CORES):
        touched, pos = place[c]
        out[c * R + touched] = res.results[c]["outs"][pos]
    return out


# revision 19
# speedup vs baseline: 1.5988x; 1.0061x over previous
"""CTDT memory updater on 8 Trainium2 NeuronCores.

Strategy (row-parallel scatter, per the node-id-range sharding):
  - The [100000, 256] memory table is sharded by node-id range across the 8
    cores (12500 rows each); weights are replicated. The host routes each
    step's (id, message) pairs to the owning core, so every step is fully
    local per core.
  - Chain routing: the host precomputes, for every (step, entry), where its
    input row value comes from: the original table (first touch of that
    row) or the output of the previous step that touched the same row.
    First-touch values are shipped from the host as a contiguous "ff"
    tensor (plain DMA, no descriptor-generation cost); repeat values are
    fetched with the SWDGE `dma_gather` custom op from a compact per-step
    output buffer `outs` in device DRAM.
  - Each step's entries are ordered [old-sourced | first-touch | recent
    (s-1)-sourced] and computed first-touch-first, so the only cross-step
    dependency (gather from outs[s-1]) resolves while independent tiles
    compute -> the PE pipeline runs continuously across steps.
  - The output table starts as a copy of the input shard; every step
    scatter-adds its per-entry delta (new - old value). Deltas telescope
    along each row's touch chain, so the final table value is exact up to
    fp32 rounding. Scatters run in the background (nothing reads the
    output table), off the critical path.
  - Matmul operands are rounded to bf16 (state, PSUM accumulation and the
    elementwise blend stay fp32): stationary-operand weight loads for
    4-byte dtypes are serial with matmuls on the PE and would dominate.

Layout notes:
  - Batch entry e = j*128 + p maps to tile j, SBUF partition p (dma_gather's
    native dst layout [128, T, elem]).
  - Gather/scatter indices are int16, wrapped in 16 partitions
    ([16, n/16] with token e at [e % 16, e // 16]) and replicated to all
    128 partitions (one copy per Q7 core).
  - Matmuls compute row-major [128b x 512] PSUM tiles: stationary operand is
    the activation chunk [k=128, b=128] (messages/dtdg pre-transposed on
    host; gathered memory rows transposed on PE), moving operand is a
    [128, 512] weight slice, plus a K=1 ones-row matmul that adds the bias.
  - gate/tanh pre-activations share one PSUM tile: columns 0:256 hold the
    sigmoid input, 256:512 the tanh input, with weight columns pre-arranged
    on the host so PSUM accumulation performs the i_C + h adds for free.
"""

import numpy as np

N_NODES = 100000
MEM = 256
N_CORES = 8
N_STEPS = 8
R = N_NODES // N_CORES  # rows per core
P = 128


def _build_program(cfg):
    import concourse.bacc as bacc
    import concourse.mybir as mybir
    import concourse.tile as tile

    f32 = mybir.dt.float32
    bf16 = mybir.dt.bfloat16
    i16 = mybir.dt.int16
    AF = mybir.ActivationFunctionType
    ALU = mybir.AluOpType

    OT, FT, RT = cfg  # per-step region tile counts
    TS = [OT[s] + FT[s] + RT[s] for s in range(N_STEPS)]
    C_MAX = max(TS) * P
    T_MAX = max(TS)
    NID_COLS = max((OT[s] + RT[s]) * 8 for s in range(N_STEPS)) or 8
    FF_ROWS = sum(FT) * P

    nc = bacc.Bacc("TRN2", target_bir_lowering=False, debug=False,
                   num_devices=N_CORES)

    ids_d = nc.dram_tensor("ids", [N_STEPS * P, NID_COLS], i16,
                           kind="ExternalInput").ap()
    ff_d = nc.dram_tensor("ff", [max(FF_ROWS, 1), MEM], f32,
                          kind="ExternalInput").ap()
    msg_d = nc.dram_tensor("msg", [N_STEPS * 2 * P, T_MAX * P], bf16,
                           kind="ExternalInput").ap()
    dt_d = nc.dram_tensor("dt", [2 * P, T_MAX * P], bf16,
                          kind="ExternalInput").ap()
    wc_d = nc.dram_tensor("wc", [2 * P, 512], bf16, kind="ExternalInput").ap()
    wh1_d = nc.dram_tensor("wh1", [2 * P, 512], bf16, kind="ExternalInput").ap()
    wh2_d = nc.dram_tensor("wh2", [2 * P, 512], bf16, kind="ExternalInput").ap()
    wd_d = nc.dram_tensor("wd", [2 * P, 512], bf16, kind="ExternalInput").ap()
    b1_d = nc.dram_tensor("b1", [1, 512], bf16, kind="ExternalInput").ap()
    b2_d = nc.dram_tensor("b2", [1, 512], bf16, kind="ExternalInput").ap()
    id_d = nc.dram_tensor("ident", [P, P], f32, kind="ExternalInput").ap()
    ones_d = nc.dram_tensor("ones1", [1, P], bf16, kind="ExternalInput").ap()
    outs_d = nc.dram_tensor("outs", [N_STEPS * C_MAX, MEM], f32,
                            kind="ExternalOutput").ap()

    def chunks_of(lo, hi, max_tiles):
        n = max(1, -(-(hi - lo) // max_tiles))
        return [(lo + ((g * (hi - lo)) // n), lo + (((g + 1) * (hi - lo)) // n))
                for g in range(n)]

    with tile.TileContext(nc) as tc:
        with (
            tc.tile_pool(name="const", bufs=1) as const_pool,
            tc.tile_pool(name="ids", bufs=2) as ids_pool,
            tc.tile_pool(name="msgr", bufs=2) as msg_pool,
            tc.tile_pool(name="mg", bufs=2) as mg_pool,
            tc.tile_pool(name="new", bufs=2) as new_pool,
            tc.tile_pool(name="work", bufs=3) as work_pool,
            tc.tile_pool(name="psum", bufs=2, space="PSUM") as psum_pool,
            tc.tile_pool(name="psumtr", bufs=2, space="PSUM") as psumtr_pool,
        ):
            # --- constants (all matmul operands arrive bf16 from the host) ---
            wc_sb = const_pool.tile([P, 1024], bf16)
            wh1_sb = const_pool.tile([P, 1024], bf16)
            wh2_sb = const_pool.tile([P, 1024], bf16)
            wd_sb = const_pool.tile([P, 1024], bf16)
            for sb, d in ((wc_sb, wc_d), (wh1_sb, wh1_d), (wh2_sb, wh2_d),
                          (wd_sb, wd_d)):
                for kc in range(2):
                    nc.sync.dma_start(out=sb[:, kc * 512:(kc + 1) * 512],
                                      in_=d[kc * P:(kc + 1) * P, :])
            b1_sb = const_pool.tile([1, 512], bf16)
            nc.sync.dma_start(out=b1_sb[:], in_=b1_d[:, :])
            b2_sb = const_pool.tile([1, 512], bf16)
            nc.sync.dma_start(out=b2_sb[:], in_=b2_d[:, :])
            ones_sb = const_pool.tile([1, P], bf16)
            nc.sync.dma_start(out=ones_sb[:], in_=ones_d[:, :])
            ident_sb = const_pool.tile([P, P], f32)
            nc.sync.dma_start(out=ident_sb[:], in_=id_d[:, :])

            ff_off = 0
            ids_tiles = []
            for s in range(N_STEPS):
                it = const_pool.tile([P, NID_COLS], i16, name=f"ids{s}")
                nc.sync.dma_start(out=it[:], in_=ids_d[s * P:(s + 1) * P, :])
                ids_tiles.append(it)

            for s in range(N_STEPS):
                final = s == N_STEPS - 1
                T = TS[s]
                ot, ft, rt = OT[s], FT[s], RT[s]
                ids_sb = ids_tiles[s]
                msg_sb = msg_pool.tile([P, 2 * T_MAX * P], bf16)
                for kc in range(2):
                    row0 = (s * 2 + kc) * P
                    nc.sync.dma_start(
                        out=msg_sb[:, kc * T_MAX * P:kc * T_MAX * P + T * P],
                        in_=msg_d[row0:row0 + P, 0:T * P])
                if final:
                    dt_sb = msg_pool.tile([P, 2 * T_MAX * P], bf16, tag="dt",
                                          bufs=1)
                    for kc in range(2):
                        nc.sync.dma_start(
                            out=dt_sb[:, kc * T_MAX * P:kc * T_MAX * P + T * P],
                            in_=dt_d[kc * P:(kc + 1) * P, 0:T * P])

                mg3 = mg_pool.tile([P, T_MAX, MEM], f32)
                # first-touch values: contiguous load from host-packed ff
                if ft:
                    nc.sync.dma_start(
                        out=mg3[:, ot:ot + ft, :],
                        in_=ff_d[ff_off:ff_off + ft * P, :].rearrange(
                            "(t p) d -> p t d", p=P))
                    ff_off += ft * P
                # old-sourced values (steps < s-1): gather from outs
                if ot:
                    for j0, j1 in chunks_of(0, ot, 8):
                        n = (j1 - j0) * P
                        nc.gpsimd.dma_gather(
                            mg3[:, j0:j1, :], outs_d[:, :],
                            ids_sb[:, j0 * 8:j1 * 8], n, n, MEM)
                # recent values (step s-1): gather from outs
                if rt:
                    for j0, j1 in chunks_of(T - rt, T, 8):
                        n = (j1 - j0) * P
                        nc.gpsimd.dma_gather(
                            mg3[:, j0:j1, :], outs_d[:, :],
                            ids_sb[:, ot * 8 + (j0 - (T - rt)) * 8:
                                   ot * 8 + (j1 - (T - rt)) * 8], n, n, MEM)

                new3 = new_pool.tile([P, T_MAX, MEM], f32)

                # compute order: first-touch tiles first (no cross-step dep),
                # then old, then recent (gather has the longest to resolve).
                # Tiles are processed in pairs within each region so one
                # 2-bank PSUM tile serves two batch tiles and ACT/DVE ops run
                # at 512 columns per instruction.
                groups = []
                for lo, hi in ((ot, ot + ft), (0, ot), (ot + ft, T)):
                    j = lo
                    while j < hi:
                        w = 2 if j + 1 < hi else 1
                        groups.append((j, w))
                        j += w
                gi_of_tile = {}
                for gi, (j0, w) in enumerate(groups):
                    for t in range(w):
                        gi_of_tile[j0 + t] = gi
                done_after = {}
                for o0, o1 in chunks_of(0, T, 8):
                    pos = max(gi_of_tile[j] for j in range(o0, o1))
                    done_after.setdefault(pos, []).append((o0, o1))

                for gi, (j0, w) in enumerate(groups):
                    mgp = mg3[:, j0:j0 + w, :]
                    tr_ps = psumtr_pool.tile([P, 2 * w * P], f32, tag="tr")
                    for t in range(w):
                        for kc in range(2):
                            nc.tensor.transpose(
                                out=tr_ps[:, (t * 2 + kc) * P:
                                          (t * 2 + kc + 1) * P],
                                in_=mg3[:, j0 + t, kc * P:(kc + 1) * P],
                                identity=ident_sb[:],
                            )
                    mgT = work_pool.tile([P, 2 * w * P], bf16, tag="mgT")
                    nc.scalar.copy(mgT[:], tr_ps[:])

                    p1 = psum_pool.tile([P, w * 512], f32, tag="p1")
                    for t in range(w):
                        j = j0 + t
                        po = t * 512
                        nc.tensor.matmul(p1[:, po:po + 512], lhsT=ones_sb[:],
                                         rhs=b1_sb[:], start=True, stop=False)
                        for kc in range(2):
                            nc.tensor.matmul(
                                p1[:, po:po + 512],
                                lhsT=msg_sb[:, kc * T_MAX * P + j * P:
                                            kc * T_MAX * P + (j + 1) * P],
                                rhs=wc_sb[:, kc * 512:(kc + 1) * 512],
                                start=False, stop=False)
                        for kc in range(2):
                            nc.tensor.matmul(
                                p1[:, po:po + 512],
                                lhsT=mgT[:, (t * 2 + kc) * P:
                                          (t * 2 + kc + 1) * P],
                                rhs=wh1_sb[:, kc * 512:(kc + 1) * 512],
                                start=False, stop=kc == 1)
                    p1t = p1[:].rearrange("p (t x) -> p t x", x=512)
                    g = work_pool.tile([P, w * MEM], f32, tag="g")
                    g3 = g[:].rearrange("p (t x) -> p t x", x=MEM)
                    nc.scalar.activation(g3, p1t[:, :, 0:MEM], AF.Sigmoid)
                    hc = work_pool.tile([P, w * MEM], f32, tag="hc")
                    hc3 = hc[:].rearrange("p (t x) -> p t x", x=MEM)
                    nc.scalar.activation(hc3, p1t[:, :, MEM:2 * MEM], AF.Tanh)

                    nw = new3[:, j0:j0 + w, :]
                    if not final:
                        # new = hc + g * (m_g - hc)
                        d_t = work_pool.tile([P, w * MEM], f32, tag="d_t")
                        d3 = d_t[:].rearrange("p (t x) -> p t x", x=MEM)
                        nc.vector.tensor_sub(d3, mgp, hc3)
                        e_t = work_pool.tile([P, w * MEM], f32, tag="e_t")
                        nc.vector.tensor_mul(e_t[:], g[:], d_t[:])
                        e3 = e_t[:].rearrange("p (t x) -> p t x", x=MEM)
                        nc.vector.tensor_add(nw, hc3, e3)
                    else:
                        p2 = psum_pool.tile([P, w * 512], f32, tag="p2",
                                            bufs=1)
                        for t in range(w):
                            j = j0 + t
                            po = t * 512
                            nc.tensor.matmul(p2[:, po:po + 512],
                                             lhsT=ones_sb[:],
                                             rhs=b2_sb[:], start=True,
                                             stop=False)
                            for kc in range(2):
                                nc.tensor.matmul(
                                    p2[:, po:po + 512],
                                    lhsT=dt_sb[:, kc * T_MAX * P + j * P:
                                               kc * T_MAX * P + (j + 1) * P],
                                    rhs=wd_sb[:, kc * 512:(kc + 1) * 512],
                                    start=False, stop=False)
                            for kc in range(2):
                                nc.tensor.matmul(
                                    p2[:, po:po + 512],
                                    lhsT=mgT[:, (t * 2 + kc) * P:
                                              (t * 2 + kc + 1) * P],
                                    rhs=wh2_sb[:, kc * 512:(kc + 1) * 512],
                                    start=False, stop=kc == 1)
                        p2t = p2[:].rearrange("p (t x) -> p t x", x=512)
                        gd = work_pool.tile([P, w * MEM], f32, tag="g2")
                        gd3 = gd[:].rearrange("p (t x) -> p t x", x=MEM)
                        nc.scalar.activation(gd3, p2t[:, :, 0:MEM], AF.Sigmoid)
                        hd = work_pool.tile([P, w * MEM], f32, tag="h2")
                        hd3 = hd[:].rearrange("p (t x) -> p t x", x=MEM)
                        nc.scalar.activation(hd3, p2t[:, :, MEM:2 * MEM],
                                             AF.Tanh)
                        # new = m_g + gd/2*(hd - m_g) + g/2*(hc - m_g)
                        d1 = work_pool.tile([P, w * MEM], f32, tag="d_t")
                        d13 = d1[:].rearrange("p (t x) -> p t x", x=MEM)
                        nc.vector.tensor_sub(d13, hd3, mgp)
                        t1 = work_pool.tile([P, w * MEM], f32, tag="e_t")
                        nc.vector.scalar_tensor_tensor(
                            t1[:], gd[:], 0.5, d1[:],
                            op0=ALU.mult, op1=ALU.mult)
                        d2 = work_pool.tile([P, w * MEM], f32, tag="d2")
                        d23 = d2[:].rearrange("p (t x) -> p t x", x=MEM)
                        nc.vector.tensor_sub(d23, hc3, mgp)
                        t2 = work_pool.tile([P, w * MEM], f32, tag="t2")
                        nc.vector.scalar_tensor_tensor(
                            t2[:], g[:], 0.5, d2[:],
                            op0=ALU.mult, op1=ALU.mult)
                        acc = work_pool.tile([P, w * MEM], f32, tag="acc")
                        nc.vector.tensor_add(acc[:], t1[:], t2[:])
                        a3 = acc[:].rearrange("p (t x) -> p t x", x=MEM)
                        nc.vector.tensor_add(nw, mgp, a3)

                    for o0, o1 in done_after.get(gi, ()):
                        # compact outs write (plain DMA, feeds later gathers
                        # and the host-side final placement)
                        a = s * C_MAX + o0 * P
                        b = s * C_MAX + o1 * P
                        nc.sync.dma_start(
                            out=outs_d[a:b, :].rearrange(
                                "(t p) d -> p t d", p=P),
                            in_=new3[:, o0:o1, :])
    nc.compile()
    return nc


def _prep_inputs(memory, node_ids, messages, dtdg_output,
                 W_C_w, W_C_b, W_D_w, W_D_b, W_h_w, W_h_b):
    import ml_dtypes

    bf16 = ml_dtypes.bfloat16
    node_ids = np.asarray(node_ids)
    owner = node_ids // R  # [8, B]

    # --- route batches to cores and classify each entry's value source ---
    percore = []
    for c in range(N_CORES):
        steps = []
        last_s = np.full(R, -1, np.int64)  # last step that touched each row
        for s in range(N_STEPS):
            sel = np.nonzero(owner[s] == c)[0]
            loc = (node_ids[s, sel] - c * R).astype(np.int64)
            prev_s = last_s[loc]
            is_ff = prev_s < 0
            is_rec = (~is_ff) & (prev_s == s - 1)
            is_old = (~is_ff) & (~is_rec)
            steps.append(dict(sel=sel, loc=loc,
                              o=np.nonzero(is_old)[0],
                              f=np.nonzero(is_ff)[0],
                              r=np.nonzero(is_rec)[0]))
            last_s[loc] = s
        percore.append(steps)

    # global region tile counts
    OT, FT, RT = [], [], []
    for s in range(N_STEPS):
        OT.append(max(-(-len(percore[c][s]["o"]) // P) for c in range(N_CORES)))
        FT.append(max(-(-len(percore[c][s]["f"]) // P) for c in range(N_CORES)))
        RT.append(max(-(-len(percore[c][s]["r"]) // P) for c in range(N_CORES)))
    cfg = (tuple(OT), tuple(FT), tuple(RT))
    TS = [OT[s] + FT[s] + RT[s] for s in range(N_STEPS)]
    C_MAX = max(TS) * P
    T_MAX = max(TS)
    NID_COLS = max((OT[s] + RT[s]) * 8 for s in range(N_STEPS)) or 8
    FF_ROWS = sum(FT) * P

    WcT = np.ascontiguousarray(W_C_w.T)
    WhT = W_h_w.T
    wh1 = np.ascontiguousarray(np.concatenate(
        [WhT[:, 256:512], WhT[:, 768:1024]], axis=1))
    wh2 = np.ascontiguousarray(np.concatenate(
        [WhT[:, 0:256], WhT[:, 512:768]], axis=1))
    WdT = np.ascontiguousarray(W_D_w.T)
    b1 = np.concatenate([W_C_b[:256] + W_h_b[256:512],
                         W_C_b[256:512] + W_h_b[768:1024]])[None, :]
    b2 = np.concatenate([W_D_b[:256] + W_h_b[0:256],
                         W_D_b[256:512] + W_h_b[512:768]])[None, :]

    def fold(block, T):
        # [T*128, 256] row-major entries -> [2*128, T*128] transposed k-major
        out = np.empty((2 * P, T_MAX * P), np.float32)
        for kc in range(2):
            b3 = block[:, kc * P:(kc + 1) * P].reshape(T, P, P)  # [j, b, k]
            out[kc * P:(kc + 1) * P, :T * P] = np.ascontiguousarray(
                b3.transpose(2, 0, 1)).reshape(P, T * P)
        return out

    def wrap16(tokens):
        # [n] -> [128, n/16] int16, token e at [e%16, e//16], replicated
        n = tokens.size
        blk = tokens.astype(np.int16).reshape(n // 16, 16).T
        return np.tile(blk, (8, 1))

    in_maps = []
    place = []   # per core: (touched_local_rows, outs_flat_positions)
    for c in range(N_CORES):
        ids_arr = np.zeros((N_STEPS * P, NID_COLS), np.int16)
        msg_arr = np.zeros((N_STEPS * 2 * P, T_MAX * P), bf16)
        dt_arr = np.zeros((2 * P, T_MAX * P), bf16)
        ff_arr = np.zeros((max(FF_ROWS, 1), MEM), np.float32)
        ff_off = 0
        # flat outs position of each (step, entry) for chain bookkeeping
        pos_of = np.full(R, -1, np.int64)
        for s in range(N_STEPS):
            st = percore[c][s]
            T = TS[s]
            C = T * P
            ot, ft, rt = OT[s], FT[s], RT[s]
            n_o, n_f, n_r = len(st["o"]), len(st["f"]), len(st["r"])

            # entry order: [old pad ot*P][ff pad ft*P][recent pad rt*P]
            perm = np.full(C, -1, np.int64)  # batch-item index per entry slot
            perm[0:n_o] = st["o"]
            perm[ot * P:ot * P + n_f] = st["f"]
            perm[(ot + ft) * P:(ot + ft) * P + n_r] = st["r"]
            valid = perm >= 0

            # gather tokens: flat outs index of each entry's source value
            if ot:
                tok = np.zeros(ot * P, np.int64)
                tok[:n_o] = pos_of[st["loc"][st["o"]]]
                assert n_o == 0 or ((tok[:n_o] >= 0).all()
                                    and (tok[:n_o] < (s - 1) * C_MAX).all())
                ids_arr[s * P:(s + 1) * P, 0:ot * 8] = wrap16(tok)
            if rt:
                tok = np.zeros(rt * P, np.int64)
                tok[:n_r] = pos_of[st["loc"][st["r"]]]
                assert n_r == 0 or ((tok[:n_r] >= (s - 1) * C_MAX).all()
                                    and (tok[:n_r] < s * C_MAX).all())
                ids_arr[s * P:(s + 1) * P, ot * 8:(ot + rt) * 8] = wrap16(tok)
            # ff values
            if ft:
                ffv = np.zeros((ft * P, MEM), np.float32)
                ffv[:n_f] = memory[c * R + st["loc"][st["f"]]]
                ff_arr[ff_off:ff_off + ft * P] = ffv
                ff_off += ft * P

            # messages / dtdg in entry order
            pm = np.zeros((C, MEM), np.float32)
            pm[valid] = messages[s, st["sel"][perm[valid]]]
            msg_arr[s * 2 * P:(s + 1) * 2 * P] = fold(pm, T).astype(bf16)
            if s == N_STEPS - 1:
                pd = np.zeros((C, MEM), np.float32)
                pd[valid] = dtdg_output[node_ids[s, st["sel"][perm[valid]]]]
                dt_arr[:] = fold(pd, T).astype(bf16)

            # record flat outs positions for the next steps' chains
            pos_of[st["loc"][perm[valid]]] = s * C_MAX + np.nonzero(valid)[0]

        touched = np.nonzero(pos_of >= 0)[0]
        place.append((touched, pos_of[touched]))
        in_maps.append(dict(
            ids=ids_arr, ff=ff_arr, msg=msg_arr, dt=dt_arr,
            wc=WcT.astype(bf16), wh1=wh1.astype(bf16),
            wh2=wh2.astype(bf16), wd=WdT.astype(bf16),
            b1=b1.astype(bf16), b2=b2.astype(bf16),
            ident=np.eye(P, dtype=np.float32),
            ones1=np.ones((1, P), bf16),
        ))
    return in_maps, cfg, place


_CACHE = {}


def run_sharded(in_maps, cfg, trace=False, **kw):
    from concourse.bass_utils import run_bass_kernel_spmd

    if cfg not in _CACHE:
        _CACHE[cfg] = _build_program(cfg)
    nc = _CACHE[cfg]
    return run_bass_kernel_spmd(nc, in_maps, core_ids=list(range(N_CORES)),
                                trace=trace, **kw)


def kernel(memory, node_ids, messages, dtdg_output,
           W_C_w, W_C_b, W_D_w, W_D_b, W_h_w, W_h_b):
    memory = np.asarray(memory, np.float32)
    messages = np.asarray(messages, np.float32)
    dtdg_output = np.asarray(dtdg_output, np.float32)
    in_maps, cfg, place = _prep_inputs(memory, node_ids, messages,
                                       dtdg_output,
                                       np.asarray(W_C_w, np.float32),
                                       np.asarray(W_C_b, np.float32),
                                       np.asarray(W_D_w, np.float32),
                                       np.asarray(W_D_b, np.float32),
                                       np.asarray(W_h_w, np.float32),
                                       np.asarray(W_h_b, np.float32))
    res = run_sharded(in_maps, cfg)
    out = np.array(memory, np.float32, copy=True)
    for c in range(N_CORES):
        touched, pos = place[c]
        out[c * R + touched] = res.results[c]["outs"][pos]
    return out
